# revision 1
# baseline (speedup 1.0000x reference)
"""BinaryNet2 MLP on 8 Trainium2 NeuronCores.

Network (reference): h = sign(matmul(sign(h), W.T)) for W0..W3 with
x [8192, 4096], W0..W2 [4096, 4096], W3 [10, 4096].

Strategy:
- Data-parallel over batch: each core gets 1024 rows, weights replicated.
- All matmul operands are in {-1, 0, +1}, so fp8(e4m3) matmuls with fp32
  PSUM accumulation are bit-exact. DoubleRow perf mode packs 2 fp8 k-rows
  per PE cell (2x ALU throughput).
- Activations kept feature-major on chip ([128 ki, 32 ksub, 1024 batch]):
  each layer's PSUM output tile [128 n, b] is directly the next layer's
  contraction input slab — zero transposes on device.
- Sign is fused into the PSUM->SBUF drain on the scalar (ACT) engine,
  writing fp8 for the next layer.
"""
import os
import sys

for _p in ("/opt/trn_rl_repo", "/root/.axon_site/_ro/trn_rl_repo"):
    if os.path.isdir(_p) and _p not in sys.path:
        sys.path.insert(0, _p)

from contextlib import ExitStack

import ml_dtypes
import numpy as np

import concourse.bass as bass
import concourse.mybir as mybir
import concourse.tile as tile
from concourse.bass_utils import run_bass_kernel_spmd
from concourse.vector_clock import ScopedClock, VectorClock

N_CORES = 8
BATCH = 8192
D = 4096
NCLS = 10
BSH = BATCH // N_CORES  # 1024 rows per core
KSUB = D // 128         # 32 k-subtiles of 128
NSUP = KSUB // 2        # 16 DoubleRow super-tiles (256 k each)
NBLK = 8                # output-feature blocks of 512
NB = D // NBLK          # 512
NT = NB // 128          # 4 n-tiles per block
HB = BSH // 512         # 2 batch halves of 512

F8 = mybir.dt.float8e4
F32 = mybir.dt.float32
f8np = ml_dtypes.float8_e4m3
DR = mybir.MatmulPerfMode.DoubleRow


def _patched_drain_and_barrier(self, tick_clock, wait_clock):
    """Waitless tail drain (walrus accepts at most one sync-wait per Drain).
    For this kernel no explicit waits are needed at all: every engine's last
    work feeds the final output DMAs, and the drain blocks until the DMA
    queues empty — which transitively covers all compute."""
    self.nc.sync.drain()
    # No closing barrier either: once the drain sees empty DMA queues, all
    # engine work has retired (it all feeds the output DMAs) and nothing
    # executes afterwards; the next run's prologue re-syncs from scratch.
    assert self.sems is not None
    popped = self.nc._tile_sem_poison_stack.pop()
    assert popped is self._sem_poison
    # Skip the exit-time dma_reset+sem_clear instructions and the second
    # barrier: the Bass prologue re-clears the whole kernel semaphore range
    # at the start of EVERY execution, so for a single re-executed NEFF the
    # exit clears only add ~4us of tail. Keep the allocator bookkeeping.
    sems = list(self.sems.allocated().values())
    sem_nums = [s.num if hasattr(s, "num") else s for s in sems]
    if sem_nums:
        self.nc._state.prepend_free_semaphores(sem_nums)
        for poison_set in self.nc._tile_sem_poison_stack:
            poison_set.update(sem_nums)


tile.TileContext._drain_and_barrier = _patched_drain_and_barrier

_orig_commit = tile.TileContext._commit_instruction


_last_ldw_key = [None]


def _ldw_key(inst):
    try:
        w = inst.ins[0]
        ap = getattr(w, "bass_ap", None)
        if ap is None:
            return None
        return (
            id(ap.tensor),
            ap.offset,
            tuple(map(tuple, ap.ap)),
            str(inst.perf_mode),
            str(getattr(inst, "tile_position", None)),
        )
    except Exception:
        return None


def _commit_split_waits(self, inst, lazy_reg_writes=True):
    """Two fixups: (1) elide LDWEIGHTS that reload the exact weights already
    in the PE array (consecutive matmuls sharing a stationary tile) — halves
    weight-path XBUS traffic; (2) walrus accepts at most one sync-wait per
    instruction, so peel extra waits onto single-wait same-engine NoOps."""
    si = getattr(inst, "sync_info", None)
    eng = getattr(inst, "engine", None)
    if type(inst).__name__ == "InstLdweights":
        clean = si is None or (not si.on_wait and not si.on_update)
        key = _ldw_key(inst)
        if clean and key is not None and key == _last_ldw_key[0]:
            # keep the name resolvable for dependency lookups, but drop the
            # instruction from the program: the PE still holds these weights
            self.nc.register_instruction(inst, overwrite=True)
            return
        _last_ldw_key[0] = key
    if (
        si is not None
        and si.on_wait
        and len(si.on_wait) > 1
        and eng is not None
        and eng != mybir.EngineType.Unassigned
    ):
        waits = list(si.on_wait)
        for w in waits[:-1]:
            nop = mybir.InstNoOp(
                name=self.nc.get_next_instruction_name(),
                sync_info=mybir.SyncInfo(on_wait=[w], on_update=[]),
                bass_nofuse=True,
                engine=eng,
            )
            _orig_commit(self, nop, lazy_reg_writes=False)
        si.on_wait = waits[-1:]
    return _orig_commit(self, inst, lazy_reg_writes)


tile.TileContext._commit_instruction = _commit_split_waits

if os.environ.get("KERNEL_LDW_OPT"):
    import concourse.bass_utils as _bu

    _orig_run_command = _bu.run_command

    def _run_command_ldwopt(argv, **kw):
        argv = [
            "--enable-ldw-opt=true" if a == "--enable-ldw-opt=false" else a
            for a in argv
        ]
        return _orig_run_command(argv, **kw)

    _bu.run_command = _run_command_ldwopt


def build_nc() -> bass.Bass:
    nc = bass.Bass()
    # g0 as 16 super-tiles so layer-0 matmuls start as soon as s=0 lands
    g0 = nc.declare_dram_parameter("g0", [NSUP, 128, 2, BSH], F8, isOutput=False)
    # weights: [nb, ki, ks, n] so each 2MB slab is one contiguous 16KB/partition
    ws = [
        nc.declare_dram_parameter(f"w{i}", [NBLK, 128, KSUB, NB], F8, isOutput=False)
        for i in range(3)
    ]
    w3 = nc.declare_dram_parameter("w3", [128, KSUB, 16], F8, isOutput=False)
    out = nc.declare_dram_parameter("out", [16, BSH], F32, isOutput=True)

    with tile.TileContext(nc) as tc, ExitStack() as ctx:
        gpool = ctx.enter_context(tc.tile_pool(name="g", bufs=1))
        wpool = ctx.enter_context(tc.tile_pool(name="w", bufs=4))
        pspool = ctx.enter_context(tc.tile_pool(name="ps", bufs=8, space="PSUM"))
        opool = ctx.enter_context(tc.tile_pool(name="o", bufs=1))

        gA = [gpool.tile([128, 2, BSH], F8, tag=f"gA{s}", name=f"gA{s}")
              for s in range(NSUP)]
        gB = [gpool.tile([128, 2, BSH], F8, tag=f"gB{s}", name=f"gB{s}")
              for s in range(NSUP)]

        def dma_slab(wt, w, nb, nsplit=4):
            # split each 2MB slab over DMA rings (one ring ~45GB/s)
            q = KSUB // nsplit
            for i in range(nsplit):
                nc.sync.dma_start(wt[:, i * q:(i + 1) * q, :], w[nb, :, i * q:(i + 1) * q, :])

        # First slab as 16 per-super-tile weight tiles, DMA-interleaved with g0
        # in consumption order: the s-outer first block below starts computing
        # after just (wt0s[0], g0[0]) land instead of the full 6MB.
        wt0s = [wpool.tile([128, 2, NB], F8, tag=f"wt0s{s}", name=f"wt0s{s}", bufs=1)
                for s in range(NSUP)]
        for s in range(NSUP):
            nc.sync.dma_start(wt0s[s][:], ws[0][0, :, 2 * s:2 * s + 2, :])
            nc.sync.dma_start(gA[s][:], g0[s])

        # warm the PE HAM clock-gate with throwaway matmuls while DMAs land
        warm = gpool.tile([128, 512], F8, tag="warm")
        nc.vector.memset(warm[:], 0.0)
        wps = pspool.tile([128, 512], F32, tag="ps", name="ps_warm")
        for i in range(12):
            nc.tensor.matmul(wps[:], warm[:, :128], warm[:], start=True, stop=True)

        gin, gout = gA, gB
        for li in range(3):
            w = ws[li]
            for nb in range(NBLK):
                if li == 0 and nb == 0:
                    # s-outer with all 8 psum tiles accumulating: MM(s) only
                    # needs (wt0s[s], g0[s]) so compute paces DMA arrival
                    ps0 = [[pspool.tile([128, 512], F32, tag="ps",
                                        name=f"ps00_{nt}_{h}")
                            for h in range(HB)] for nt in range(NT)]
                    for s in range(NSUP):
                        for nt in range(NT):
                            for h in range(HB):
                                nc.tensor.matmul(
                                    ps0[nt][h][:],
                                    wt0s[s][:, :, nt * 128:(nt + 1) * 128],
                                    gin[s][:, :, h * 512:(h + 1) * 512],
                                    start=(s == 0),
                                    stop=(s == NSUP - 1),
                                    perf_mode=DR,
                                )
                    for nt in range(NT):
                        for h in range(HB):
                            nc.scalar.sign(
                                gout[nt // 2][:, nt % 2, h * 512:(h + 1) * 512],
                                ps0[nt][h][:],
                            )
                    continue
                wt = wpool.tile([128, KSUB, NB], F8, tag="wt", name=f"wt_{li}_{nb}")
                dma_slab(wt, w, nb)
                for nt in range(NT):
                    # h innermost: each stationary weight tile feeds both
                    # batch halves, so LDWEIGHTS amortizes over 2 matmuls
                    pss = [pspool.tile([128, 512], F32, tag="ps", name=f"ps_{nb}_{nt}_{h}")
                           for h in range(HB)]
                    for s in range(NSUP):
                        for h in range(HB):
                            nc.tensor.matmul(
                                pss[h][:],
                                wt[:, 2 * s:2 * s + 2, nt * 128:(nt + 1) * 128],
                                gin[s][:, :, h * 512:(h + 1) * 512],
                                start=(s == 0),
                                stop=(s == NSUP - 1),
                                perf_mode=DR,
                            )
                    t = nb * NT + nt  # output feature tile -> (super, slot)
                    for h in range(HB):
                        nc.scalar.sign(
                            gout[t // 2][:, t % 2, h * 512:(h + 1) * 512], pss[h][:]
                        )
            gin, gout = gout, gin

        # final layer: [10, 4096] weights (tiny)
        w3t = wpool.tile([128, KSUB, 16], F8, tag="w3")
        nc.sync.dma_start(w3t[:], w3[:])
        ot = opool.tile([16, BSH], F32, tag="ot")
        for h in range(HB):
            ps = pspool.tile([128, 512], F32, tag="ps", name=f"ps3_{h}")
            for s in range(NSUP):
                nc.tensor.matmul(
                    ps[:16, :],
                    w3t[:, 2 * s:2 * s + 2, :],
                    gin[s][:, :, h * 512:(h + 1) * 512],
                    start=(s == 0),
                    stop=(s == NSUP - 1),
                    perf_mode=DR,
                )
            # sign + store of half h overlap the other half's matmuls
            nc.scalar.sign(ot[:, h * 512:(h + 1) * 512], ps[:16, :])
            nc.sync.dma_start(out[:, h * 512:(h + 1) * 512],
                              ot[:, h * 512:(h + 1) * 512])
    return nc


_NC_CACHE: list = []


def _get_nc() -> bass.Bass:
    if not _NC_CACHE:
        _NC_CACHE.append(build_nc())
    return _NC_CACHE[0]


def _prep_weight(W: np.ndarray) -> np.ndarray:
    """[4096, 4096] f32 -> [NBLK nb, 128 ki, KSUB ks, NB nj] fp8,
    w[nb, ki, ks, nj] = W.T[ks*128 + ki, nb*512 + nj]."""
    WT = W.astype(np.float32).T  # [k, n]
    t = WT.reshape(KSUB, 128, NBLK, NB).transpose(2, 1, 0, 3)
    return np.ascontiguousarray(t).astype(f8np)


def _prep_w3(W3: np.ndarray) -> np.ndarray:
    """[10, 4096] f32 -> [128 ki, KSUB ks, 16] fp8 (padded classes)."""
    W3p = np.zeros((16, D), np.float32)
    W3p[:NCLS] = np.asarray(W3, dtype=np.float32)
    t = W3p.T.reshape(KSUB, 128, 16).transpose(1, 0, 2)
    return np.ascontiguousarray(t).astype(f8np)


LAST_EXEC_NS = [None]


def _install_ntff_shim():
    """The image's antenv package lacks axon_hooks; provide it so
    run_bass_kernel_spmd(trace=True) can reach the terminal's NTFF capture."""
    import types

    if "antenv.axon_hooks" in sys.modules:
        return
    mod = types.ModuleType("antenv.axon_hooks")
    holder = [None]
    mod.set_axon_ntff_profile_hook = lambda h: holder.__setitem__(0, h)
    mod.get_axon_ntff_profile_hook = lambda: holder[0]
    sys.modules["antenv.axon_hooks"] = mod
    try:
        import trn_agent_boot.trn_boot as tb

        holder[0] = tb._ntff_profile_via_ctypes("/opt/axon/libaxon_pjrt.so")
    except Exception as e:  # degrade to no tracing
        print(f"ntff shim install failed: {e}", file=sys.stderr)


def kernel(x, W0, W1, W2, W3):
    x = np.asarray(x, dtype=np.float32)
    nc = _get_nc()

    w_args = {f"w{i}": _prep_weight(W) for i, W in enumerate((W0, W1, W2))}
    w_args["w3"] = _prep_w3(W3)

    in_maps = []
    for c in range(N_CORES):
        xs = x[c * BSH:(c + 1) * BSH]  # [1024, 4096]
        # g0[s, ki, j, b] = sign(x)[b, (2s+j)*128 + ki]
        g = np.sign(xs).T.reshape(NSUP, 2, 128, BSH).transpose(0, 2, 1, 3)
        in_maps.append({"g0": np.ascontiguousarray(g).astype(f8np), **w_args})

    trace = bool(os.environ.get("KERNEL_TRACE"))
    if trace:
        _install_ntff_shim()
    r = run_bass_kernel_spmd(nc, in_maps, list(range(N_CORES)), trace=trace)
    LAST_EXEC_NS[0] = r.exec_time_ns
    if trace and r.exec_time_ns is not None:
        print(f"HW exec time: {r.exec_time_ns} ns")
        if r.instructions_and_trace is not None:
            print(f"trace: {r.instructions_and_trace[1]}")

    out = np.empty((BATCH, NCLS), np.float32)
    for c in range(N_CORES):
        out[c * BSH:(c + 1) * BSH] = r.results[c]["out"][:NCLS].T
    return out



# revision 4
# speedup vs baseline: 1.1060x; 1.1060x over previous
"""BinaryNet2 MLP on 8 Trainium2 NeuronCores — Winograd/Strassen variant.

Network (reference): h = sign(matmul(sign(h), W.T)) for W0..W3 with
x [8192, 4096], W0..W2 [4096, 4096], W3 [10, 4096].

Strategy:
- Data-parallel over batch: each core gets 1024 rows, weights replicated.
- All operands are small integers, so fp8(e4m3) matmuls with fp32 PSUM
  accumulation are bit-exact. DoubleRow perf mode packs 2 fp8 k-rows per
  PE cell (2x ALU throughput).
- Each 4096x4096 layer runs one level of Winograd's 7-multiplication
  Strassen form: C blocks [2048 n, 512 b], contraction 2048. This is
  EXACT here: block sums stay tiny integers (|.|<=4, exact in fp8e4) and
  all products/sums are integers < 2^24 (exact in fp32). Tensor-engine
  work drops 8/7 -> 12.5% fewer matmul cycles.
- Weight-side Winograd combos are precomputed on host (free).
  Activation-side combos (T1..T4) and the 7 post-adds run on the idle
  Vector engine; sign() on the Scalar engine, overlapped with matmuls.
- Layer output n-subtile j directly produces the next layer's
  contraction subtile j for all 7 rhs operands — no transposes anywhere.
"""
import os
import sys

for _p in ("/opt/trn_rl_repo", "/root/.axon_site/_ro/trn_rl_repo"):
    if os.path.isdir(_p) and _p not in sys.path:
        sys.path.insert(0, _p)

from contextlib import ExitStack

import ml_dtypes
import numpy as np

import concourse.bass as bass
import concourse.mybir as mybir
import concourse.tile as tile
from concourse.bass_utils import run_bass_kernel_spmd

N_CORES = 8
BATCH = 8192
D = 4096
NCLS = 10
BSH = BATCH // N_CORES  # 1024 rows per core
HB = 512                # b-half (Strassen block column)
KH = D // 2             # 2048 k-half
KS = KH // 128          # 16 k-subtiles of 128 per product
NSUP = KS // 2          # 8 DoubleRow super-tiles per product
NJ = KH // 128          # 16 n-subtiles of 128 per product

F8 = mybir.dt.float8e4
F32 = mybir.dt.float32
f8np = ml_dtypes.float8_e4m3
DR = mybir.MatmulPerfMode.DoubleRow

# P-product rhs operands (buffer slot per product):
#   P1=A11*B11  P2=A12*B21  P3=S4*B22  P4=A22*T4  P5=S1*T1  P6=S2*T2  P7=S3*T3
# slots: 0=B11 1=B21 2=B22 3=T4 4=T1 5=T2 6=T3
RHS_OF = [0, 1, 2, 3, 4, 5, 6]
# MM emission order per n-subtile (lets DVE combines start early and
# free PSUM banks in allocation order): P1 P6 P2 P7 P5 P3 P4
MMORD = [0, 5, 1, 6, 4, 2, 3]


def _patched_drain_and_barrier(self, tick_clock, wait_clock):
    """Waitless tail drain (walrus accepts at most one sync-wait per Drain).
    Every engine's last work feeds the final output DMAs, and the drain
    blocks until the DMA queues empty — which transitively covers all
    compute."""
    self.nc.sync.drain()
    assert self.sems is not None
    popped = self.nc._tile_sem_poison_stack.pop()
    assert popped is self._sem_poison
    sems = list(self.sems.allocated().values())
    sem_nums = [s.num if hasattr(s, "num") else s for s in sems]
    if sem_nums:
        self.nc._state.prepend_free_semaphores(sem_nums)
        for poison_set in self.nc._tile_sem_poison_stack:
            poison_set.update(sem_nums)


tile.TileContext._drain_and_barrier = _patched_drain_and_barrier

_orig_commit = tile.TileContext._commit_instruction


def _commit_split_waits(self, inst, lazy_reg_writes=True):
    """walrus accepts at most one sync-wait per instruction; peel extra
    waits onto single-wait same-engine NoOps."""
    si = getattr(inst, "sync_info", None)
    eng = getattr(inst, "engine", None)
    if (
        si is not None
        and si.on_wait
        and len(si.on_wait) > 1
        and eng is not None
        and eng != mybir.EngineType.Unassigned
    ):
        waits = list(si.on_wait)
        for w in waits[:-1]:
            nop = mybir.InstNoOp(
                name=self.nc.get_next_instruction_name(),
                sync_info=mybir.SyncInfo(on_wait=[w], on_update=[]),
                bass_nofuse=True,
                engine=eng,
            )
            _orig_commit(self, nop, lazy_reg_writes=False)
        si.on_wait = waits[-1:]
    return _orig_commit(self, inst, lazy_reg_writes)


tile.TileContext._commit_instruction = _commit_split_waits


def build_nc() -> bass.Bass:
    nc = bass.Bass()
    # layer-0 rhs operands, one [128 ki, 16 ks, 512 b] buffer per product
    g0 = nc.declare_dram_parameter("g0", [7, 128, KS, HB], F8, isOutput=False)
    # per layer: 7 stationary Winograd operands, [i, j, ki, ks, n]
    # w[l][i, j, ki, ks, n] = S_i[j*128+n, ks*128+ki]
    ws = [
        nc.declare_dram_parameter(f"w{l}", [7, NJ, 128, KS, 128], F8, isOutput=False)
        for l in range(3)
    ]
    w3 = nc.declare_dram_parameter("w3", [128, 2 * KS, 16], F8, isOutput=False)
    out = nc.declare_dram_parameter("out", [16, BSH], F32, isOutput=True)

    with tile.TileContext(nc) as tc, ExitStack() as ctx:
        gpool = ctx.enter_context(tc.tile_pool(name="g", bufs=1))
        wpool = ctx.enter_context(tc.tile_pool(name="w", bufs=16))
        pspool = ctx.enter_context(tc.tile_pool(name="ps", bufs=8, space="PSUM"))
        upool = ctx.enter_context(tc.tile_pool(name="u", bufs=8))
        cpool = ctx.enter_context(tc.tile_pool(name="c", bufs=6))
        spool = ctx.enter_context(tc.tile_pool(name="s", bufs=2))
        opool = ctx.enter_context(tc.tile_pool(name="o", bufs=1))

        gA = [gpool.tile([128, KS, HB], F8, tag=f"gA{i}", name=f"gA{i}")
              for i in range(7)]
        gB = [gpool.tile([128, KS, HB], F8, tag=f"gB{i}", name=f"gB{i}")
              for i in range(7)]

        # ---- layer-0 input + first-subtile weights, in consumption order
        wt = {}

        def fetch_w(l, j):
            for i in MMORD:
                t = wpool.tile([128, KS, 128], F8, tag="wt", name=f"wt_{l}_{j}_{i}")
                nc.sync.dma_start(t[:], ws[l][i, j])
                wt[(l, j, i)] = t

        for i in MMORD:
            # rhs buffer for product i in 4 chunks (2KB/partition each)
            for q in range(4):
                nc.sync.dma_start(
                    gA[RHS_OF[i]][:, 4 * q:4 * q + 4, :],
                    g0[RHS_OF[i], :, 4 * q:4 * q + 4, :],
                )
            t = wpool.tile([128, KS, 128], F8, tag="wt", name=f"wt_0_0_{i}")
            nc.sync.dma_start(t[:], ws[0][i, 0])
            wt[(0, 0, i)] = t

        # warm the PE HAM clock-gate with throwaway matmuls while DMAs land
        warm = gpool.tile([128, 512], F8, tag="warm")
        nc.vector.memset(warm[:], 0.0)
        wps = pspool.tile([128, 512], F32, tag="ps", name="ps_warm")
        for i in range(12):
            nc.tensor.matmul(wps[:], warm[:, :128], warm[:], start=True, stop=True)

        gin, gout = gA, gB
        for l in range(3):
            for j in range(NJ):
                # prefetch next subtile's weights (or next layer's j=0)
                if j + 1 < NJ:
                    fetch_w(l, j + 1)
                elif l + 1 < 3:
                    fetch_w(l + 1, 0)

                def mm(i, ps=None):
                    first = ps is None
                    if first:
                        ps = pspool.tile([128, 512], F32, tag="ps",
                                         name=f"ps_{l}_{j}_{i}")
                    w_t = wt.pop((l, j, i))
                    g_t = gin[RHS_OF[i]]
                    for s in range(NSUP):
                        nc.tensor.matmul(
                            ps[:],
                            w_t[:, 2 * s:2 * s + 2, :],
                            g_t[:, 2 * s:2 * s + 2, :],
                            start=(first and s == 0),
                            stop=(s == NSUP - 1),
                            perf_mode=DR,
                        )
                    return ps

                p1 = mm(0)
                p6 = mm(5)
                # snapshot P1 to SBUF (ACT), then accumulate P2 onto its
                # bank: the bank becomes C11 = P1+P2 with no DVE work.
                p1s = upool.tile([128, 512], F32, tag="u", name=f"p1s_{l}_{j}")
                nc.scalar.copy(p1s[:], p1[:])
                u1 = upool.tile([128, 512], F32, tag="u", name=f"u1_{l}_{j}")
                nc.vector.tensor_add(u1[:], p1s[:], p6[:])
                mm(1, ps=p1)
                nc.scalar.sign(gout[0][:, j, :], p1[:])
                p7 = mm(6)
                u2 = upool.tile([128, 512], F32, tag="u", name=f"u2_{l}_{j}")
                nc.vector.tensor_add(u2[:], u1[:], p7[:])
                p5 = mm(4)
                u3 = upool.tile([128, 512], F32, tag="u", name=f"u3_{l}_{j}")
                nc.vector.tensor_add(u3[:], u1[:], p5[:])
                c22 = cpool.tile([128, 512], F32, tag="c", name=f"c22_{l}_{j}")
                nc.vector.tensor_add(c22[:], u2[:], p5[:])
                nc.scalar.sign(gout[2][:, j, :], c22[:])
                p3 = mm(2)
                c12 = cpool.tile([128, 512], F32, tag="c", name=f"c12_{l}_{j}")
                nc.vector.tensor_add(c12[:], u3[:], p3[:])
                if l == 2:
                    s12 = gout[3][:, j, :]
                else:
                    s12t = spool.tile([128, 512], F8, tag="s12", name=f"s12_{l}_{j}")
                    s12 = s12t[:]
                nc.scalar.sign(s12, c12[:])
                p4 = mm(3)
                c21 = cpool.tile([128, 512], F32, tag="c", name=f"c21_{l}_{j}")
                nc.vector.tensor_sub(c21[:], u2[:], p4[:])
                nc.scalar.sign(gout[1][:, j, :], c21[:])
                if l < 2:
                    # next layer's T combos: T1=s12-s11 T2=s22-T1
                    # T3=s22-s12 T4=T2-s21
                    nc.vector.tensor_sub(gout[4][:, j, :], s12, gout[0][:, j, :])
                    nc.vector.tensor_sub(gout[5][:, j, :], gout[2][:, j, :],
                                         gout[4][:, j, :])
                    nc.vector.tensor_sub(gout[6][:, j, :], gout[2][:, j, :], s12)
                    nc.vector.tensor_sub(gout[3][:, j, :], gout[5][:, j, :],
                                         gout[1][:, j, :])
            gin, gout = gout, gin

        # final layer: [10, 4096] weights (tiny). gin slots after 3 swaps:
        # gB holds {0: B11, 1: B21, 2: B22, 3: B12} of the last activation.
        w3t = wpool.tile([128, 2 * KS, 16], F8, tag="w3")
        nc.sync.dma_start(w3t[:], w3[:])
        ot = opool.tile([16, BSH], F32, tag="ot")
        for h in range(2):
            lo, hi = (0, 1) if h == 0 else (3, 2)  # k-half0, k-half1 buffers
            ps = pspool.tile([128, 512], F32, tag="ps", name=f"ps3_{h}")
            for s in range(2 * NSUP):
                g_t = gin[lo] if s < NSUP else gin[hi]
                ss = s if s < NSUP else s - NSUP
                nc.tensor.matmul(
                    ps[:16, :],
                    w3t[:, 2 * s:2 * s + 2, :],
                    g_t[:, 2 * ss:2 * ss + 2, :],
                    start=(s == 0),
                    stop=(s == 2 * NSUP - 1),
                    perf_mode=DR,
                )
            nc.scalar.sign(ot[:, h * 512:(h + 1) * 512], ps[:16, :])
            nc.sync.dma_start(out[:, h * 512:(h + 1) * 512],
                              ot[:, h * 512:(h + 1) * 512])
    return nc


_NC_CACHE: list = []


def _get_nc() -> bass.Bass:
    if not _NC_CACHE:
        _NC_CACHE.append(build_nc())
    return _NC_CACHE[0]


def _prep_weight(W: np.ndarray) -> np.ndarray:
    """[4096, 4096] f32 -> [7, NJ j, 128 ki, KS ks, 128 n] fp8 Winograd
    stationary operands; w[i, j, ki, ks, n] = S_i[j*128+n, ks*128+ki]."""
    W = np.asarray(W, dtype=np.float32)
    A11, A12 = W[:KH, :KH], W[:KH, KH:]
    A21, A22 = W[KH:, :KH], W[KH:, KH:]
    S1 = A21 + A22
    S2 = S1 - A11
    S3 = A11 - A21
    S4 = A12 - S2
    SW = [A11, A12, S4, A22, S1, S2, S3]  # P1..P7 stationary operands
    outw = np.empty((7, NJ, 128, KS, 128), dtype=f8np)
    for i, S in enumerate(SW):
        t = S.T.reshape(KS, 128, NJ, 128).transpose(2, 1, 0, 3)
        outw[i] = np.ascontiguousarray(t).astype(f8np)
    return outw


def _prep_w3(W3: np.ndarray) -> np.ndarray:
    """[10, 4096] f32 -> [128 ki, 2*KS ks, 16] fp8 (padded classes)."""
    W3p = np.zeros((16, D), np.float32)
    W3p[:NCLS] = np.asarray(W3, dtype=np.float32)
    t = W3p.T.reshape(2 * KS, 128, 16).transpose(1, 0, 2)
    return np.ascontiguousarray(t).astype(f8np)


def _prep_g0(xs: np.ndarray) -> np.ndarray:
    """[1024, 4096] f32 -> [7, 128 ki, KS ks, 512 b] fp8 rhs operands
    {B11, B21, B22, T4, T1, T2, T3} of sign(xs).T."""
    gT = np.sign(xs.astype(np.float32)).T  # [4096 k, 1024 b]
    B11, B12 = gT[:KH, :HB], gT[:KH, HB:]
    B21, B22 = gT[KH:, :HB], gT[KH:, HB:]
    T1 = B12 - B11
    T2 = B22 - T1
    T3 = B22 - B12
    T4 = T2 - B21
    blocks = [B11, B21, B22, T4, T1, T2, T3]
    g = np.empty((7, 128, KS, HB), dtype=f8np)
    for i, blk in enumerate(blocks):
        # buf[ki, ks, b] = blk[ks*128+ki, b]
        g[i] = blk.reshape(KS, 128, HB).transpose(1, 0, 2).astype(f8np)
    return g


LAST_EXEC_NS = [None]


def _install_ntff_shim():
    """The image's antenv package lacks axon_hooks; provide it so
    run_bass_kernel_spmd(trace=True) can reach the terminal's NTFF capture."""
    import types

    if "antenv.axon_hooks" in sys.modules:
        return
    mod = types.ModuleType("antenv.axon_hooks")
    holder = [None]
    mod.set_axon_ntff_profile_hook = lambda h: holder.__setitem__(0, h)
    mod.get_axon_ntff_profile_hook = lambda: holder[0]
    sys.modules["antenv.axon_hooks"] = mod
    try:
        import trn_agent_boot.trn_boot as tb

        holder[0] = tb._ntff_profile_via_ctypes("/opt/axon/libaxon_pjrt.so")
    except Exception as e:  # degrade to no tracing
        print(f"ntff shim install failed: {e}", file=sys.stderr)


def kernel(x, W0, W1, W2, W3):
    x = np.asarray(x, dtype=np.float32)
    nc = _get_nc()

    w_args = {f"w{i}": _prep_weight(W) for i, W in enumerate((W0, W1, W2))}
    w_args["w3"] = _prep_w3(W3)

    in_maps = []
    for c in range(N_CORES):
        xs = x[c * BSH:(c + 1) * BSH]  # [1024, 4096]
        in_maps.append({"g0": _prep_g0(xs), **w_args})

    trace = bool(os.environ.get("KERNEL_TRACE"))
    if trace:
        _install_ntff_shim()
    r = run_bass_kernel_spmd(nc, in_maps, list(range(N_CORES)), trace=trace)
    LAST_EXEC_NS[0] = r.exec_time_ns
    if trace and r.exec_time_ns is not None:
        print(f"HW exec time: {r.exec_time_ns} ns")
        if r.instructions_and_trace is not None:
            print(f"trace: {r.instructions_and_trace[1]}")

    out = np.empty((BATCH, NCLS), np.float32)
    for c in range(N_CORES):
        out[c * BSH:(c + 1) * BSH] = r.results[c]["out"][:NCLS].T
    return out
